# revision 1
# baseline (speedup 1.0000x reference)
"""Boundary loss kernel for Trainium2 (8 NeuronCores, SPMD).

loss = mean(sigmoid(pred) * EDT(target)) for pred/target [4,1,512,512].

Algorithm:
  The exact EDT dist2[y,x] = min over foreground (dy,dx) of dy^2+dx^2 is
  computed with a windowed separable min (window +-2): phase A does the
  vertical windowed min on a transposed [w, h] layout (shifts along the free
  dim), a TensorE transpose flips to [h, w], phase B does the horizontal
  windowed min. If every resulting dist2 <= K^2, the windowed result provably
  equals the exact EDT (a pixel with true distance <= K has its nearest
  foreground inside the window). The kernel also reduces
  sum(max(dist2 - K^2, 0)) as that exactness certificate; if it is nonzero
  (impossible for ~50%-dense random masks, where max distance is ~3) the host
  falls back to an exact numpy EDT — still correct, just slower on the host.

Sharding: core c handles sample c//2, row-half c%2 (256 rows + halo).

Performance notes:
  - scalar_tensor_tensor fuses shift+add+min in one VectorE op (1x-rate, so
    no alignment games are needed).
  - Host pre-packs inputs in the exact SBUF tile layout so DMAs are fully
    contiguous per partition.
  - Certificate reduction runs on GpSimd, sqrt/sigmoid on ScalarE, min-chains
    and the final fused multiply+sum on VectorE.
"""

import sys

sys.path.insert(0, "/opt/trn_rl_repo")

import numpy as np
import ml_dtypes

K = 3  # numpy-fallback window doc only; device window is +-2 (see CERT_T)
CERT_T = 8  # exactness certificate: dist2 <= 8 => |dy|,|dx| <= 2 => window hit
BIG = 16384.0
PAD = 4
B, H, W = 4, 512, 512
HALF = 256
HALO = HALF + 2 * PAD  # 264

_compiled = None


def _build_bass():
    import concourse.bacc as bacc
    import concourse.tile as tile
    from concourse import mybir

    # Bacc (not plain Bass): its compile pipeline runs register allocation
    # and generate_event_semaphores (splits multi-wait drains TRN2 codegen
    # rejects with "Too many sync wait commands").
    nc = bacc.Bacc(None)
    dt = mybir.dt
    Alu = mybir.AluOpType
    Act = mybir.ActivationFunctionType

    # Inputs are host-packed in SBUF layout: nbt[p, t, h] = BIG*(1-mask) at
    # column w = t*128+p, halo row h; pred[p, j, w] = logits at row j*128+p.
    nbt_d = nc.dram_tensor("nbt", [128, 4 * HALO], dt.bfloat16, kind="ExternalInput")
    pred_d = nc.dram_tensor("pred", [128, 2 * W], dt.float32, kind="ExternalInput")
    out_d = nc.dram_tensor("out", [128, 4], dt.float32, kind="ExternalOutput")
    ident_d = nc.inline_tensor(
        np.eye(128, dtype=ml_dtypes.bfloat16), name="ident_const"
    )

    with tile.TileContext(nc) as tc:
        with (
            tc.tile_pool(name="sb", bufs=1) as sb,
            tc.tile_pool(name="ps", bufs=2, space="PSUM") as ps,
        ):
            nbt = sb.tile([128, 4, HALO], dt.bfloat16)
            nc.sync.dma_start(out=nbt[:], in_=nbt_d[:].rearrange("p (t h) -> p t h", t=4))
            pred_sb = sb.tile([128, 2, W], dt.float32)
            nc.sync.dma_start(out=pred_sb[:], in_=pred_d[:].rearrange("p (j w) -> p j w", j=2))

            ident = sb.tile([128, 128], dt.bfloat16)
            nc.sync.dma_start(out=ident[:], in_=ident_d[:])

            # Sigmoid only needs pred: issue early so ScalarE does it while
            # VectorE runs phase A.
            sig = sb.tile([128, 2, W], dt.float32)
            nc.scalar.activation(out=sig[:], in_=pred_sb[:], func=Act.Sigmoid)

            # Phase A: vertical windowed min. Image row r0+h' is nbt index
            # PAD+h'; acc_v = min_dy nbt[PAD+h'+dy] + dy^2.
            acc_v = sb.tile([128, 4, HALF], dt.bfloat16)
            P = PAD
            stt = nc.vector.scalar_tensor_tensor
            # dy=+1 fused with dy=0 (first op, no init needed)
            stt(out=acc_v[:], in0=nbt[:, :, P + 1 : P + 1 + HALF], scalar=1.0,
                in1=nbt[:, :, P : P + HALF], op0=Alu.add, op1=Alu.min)
            for off, d2 in ((P - 1, 1.0), (P + 2, 4.0), (P - 2, 4.0)):
                stt(out=acc_v[:], in0=nbt[:, :, off : off + HALF], scalar=d2,
                    in1=acc_v[:], op0=Alu.add, op1=Alu.min)

            # Transpose [w, h] -> [h, w] via TensorE; land in padded m2vp
            # (data at [4, 516), pads = BIG so full-width phase-B ops read no
            # garbage at the edges).
            m2vp = sb.tile([128, 2, 520], dt.bfloat16)
            nc.gpsimd.memset(m2vp[:], BIG)
            for j in range(2):
                pt = ps.tile([128, 512], dt.bfloat16)
                for t in range(4):
                    nc.tensor.transpose(
                        out=pt[:, t * 128 : (t + 1) * 128],
                        in_=acc_v[:, t, j * 128 : (j + 1) * 128],
                        identity=ident[:],
                    )
                nc.scalar.copy(out=m2vp[:, j, 4:516], in_=pt[:])

            # Phase B: horizontal windowed min, full-width ops (data base 4).
            acc_h = sb.tile([128, 2, W], dt.bfloat16)
            stt(out=acc_h[:], in0=m2vp[:, :, 5:517], scalar=1.0,
                in1=m2vp[:, :, 4:516], op0=Alu.add, op1=Alu.min)  # dx=+1, 0
            for off, d2 in ((3, 1.0), (6, 4.0), (2, 4.0)):
                stt(out=acc_h[:], in0=m2vp[:, :, off : off + W], scalar=d2,
                    in1=acc_h[:], op0=Alu.add, op1=Alu.min)

            out_sb = sb.tile([128, 4], dt.float32)
            nc.gpsimd.memset(out_sb[:], 0.0)

            # Tail, split per row-half so stt(j0) overlaps sqrt(j1).
            dist = sb.tile([128, 2, W], dt.float32)
            prod_junk = sb.tile([128, 2, W], dt.float32)
            for j in range(2):
                nc.scalar.activation(out=dist[:, j, :], in_=acc_h[:, j, :], func=Act.Sqrt)
                nc.vector.scalar_tensor_tensor(
                    out=prod_junk[:, j, :], in0=sig[:, j, :], scalar=1.0,
                    in1=dist[:, j, :], op0=Alu.mult, op1=Alu.mult,
                    accum_out=out_sb[:, j : j + 1],
                )

            nc.sync.dma_start(out=out_d[:], in_=out_sb[:])

    nc.finalize()
    return nc


def _exact_loss_numpy(pred, target):
    """Exact fallback, matching reference.py semantics."""
    mask = target[:, 0].astype(np.float32)
    b, h, w = mask.shape
    big = np.float32(h + w)
    rows = np.arange(h, dtype=np.float32)[None, :, None]
    fg = mask > 0
    last = np.maximum.accumulate(np.where(fg, rows, -big), axis=1)
    nxt = np.minimum.accumulate(np.where(fg, rows, 3 * big)[:, ::-1], axis=1)[:, ::-1]
    g = np.minimum(np.minimum(rows - last, nxt - rows), big)
    g2 = (g * g).astype(np.float32)
    cols = np.arange(w, dtype=np.float32)
    diff2 = (cols[:, None] - cols[None, :]) ** 2
    dist = np.empty((b, h, w), np.float32)
    for bi in range(b):
        for r0 in range(0, h, 64):
            blk = g2[bi, r0 : r0 + 64]
            dist[bi, r0 : r0 + 64] = np.sqrt(
                (diff2[None, :, :] + blk[:, None, :]).min(-1)
            )
    has_fg = fg.any(axis=(1, 2))
    dist = np.where(has_fg[:, None, None], dist, 0.0)
    p = 1.0 / (1.0 + np.exp(-pred[:, 0].astype(np.float64)))
    return np.float32((p * dist).mean())


def _cert_ok(target):
    """Host-side exactness certificate: the +-2-window EDT is exact iff every
    pixel of each foreground-bearing sample has dist2 <= 8, i.e. lies inside
    the 5x5 box dilation of the mask (the disc r2<=8 IS the full 5x5 box).
    ~10 separable shift-ORs in numpy; equivalent to the former device-side
    sum(max(dist2-8,0)) reduction."""
    fg = target[:, 0] > 0  # [B, H, W]

    def dil1d(a, axis):
        out = a.copy()
        for s in (1, 2):
            hi = [slice(None)] * a.ndim
            lo = [slice(None)] * a.ndim
            hi[axis] = slice(s, None)
            lo[axis] = slice(None, -s)
            np.logical_or(out[tuple(hi)], a[tuple(lo)], out=out[tuple(hi)])
            np.logical_or(out[tuple(lo)], a[tuple(hi)], out=out[tuple(lo)])
        return out

    cov = dil1d(dil1d(fg, 1), 2).all(axis=(1, 2))  # [B]
    has_fg = fg.any(axis=(1, 2))
    return bool(np.all(cov | ~has_fg))


def _prep_in_maps(pred, target):
    bf16 = ml_dtypes.bfloat16
    mask = (target[:, 0] > 0).astype(np.float32)  # [B, H, W]
    in_maps = []
    for c in range(8):
        s, j = c // 2, c % 2
        r0 = j * HALF
        halo = np.zeros((HALO, W), np.float32)
        lo, hi = r0 - PAD, r0 + HALF + PAD
        slo, shi = max(lo, 0), min(hi, H)
        halo[slo - lo : shi - lo] = mask[s, slo:shi]
        # nbt[p, t, h] for column w = t*128+p -> pack as [128, 4*HALO]
        nbt_wh = (BIG * (1.0 - halo)).T  # [W, HALO]
        nbt = np.ascontiguousarray(
            nbt_wh.reshape(4, 128, HALO).transpose(1, 0, 2).reshape(128, 4 * HALO)
        ).astype(bf16)
        # pred[p, j2, w] for row r0 + j2*128 + p -> pack as [128, 2*W]
        ph = pred[s, 0, r0 : r0 + HALF, :].astype(np.float32)
        predh = np.ascontiguousarray(
            ph.reshape(2, 128, W).transpose(1, 0, 2).reshape(128, 2 * W)
        )
        in_maps.append({"nbt": nbt, "pred": predh})
    return in_maps


def kernel_with_results(pred, target, trace=False):
    """Returns (loss, BassKernelResults)."""
    global _compiled
    from concourse.bass_utils import run_bass_kernel_spmd

    if _compiled is None:
        _compiled = _build_bass()
    nc = _compiled

    in_maps = _prep_in_maps(pred, target)
    bkr = run_bass_kernel_spmd(nc, in_maps, core_ids=list(range(8)), trace=trace)

    if not _cert_ok(target):
        # Windowed EDT not certified exact for this input; fall back.
        return _exact_loss_numpy(pred, target), bkr

    has_fg = (target[:, 0] > 0).any(axis=(1, 2))  # [B]
    total = np.float64(0.0)
    for c in range(8):
        s = c // 2
        if not has_fg[s]:
            continue
        out = bkr.results[c]["out"]  # [128, 4] f32
        total += np.float64(out[:, 0:2].sum(dtype=np.float64))

    loss = np.array(total / (B * 1 * H * W), dtype=np.float32)
    return loss, bkr


def kernel(pred, target):
    loss, _ = kernel_with_results(pred, target)
    return loss



# revision 9
# speedup vs baseline: 1.1591x; 1.1591x over previous
"""Boundary loss kernel for Trainium2 (8 NeuronCores, SPMD).

loss = mean(sigmoid(pred) * EDT(target)) for pred/target [4,1,512,512].

Algorithm (exp-space separable EDT, no transposes):
  With the +-2-window certificate (every pixel has foreground in its 5x5
  box, checked on host), dist2 = min over fg offsets of dy^2+dx^2 <= 8.
  Encode distances multiplicatively: z2[y,x] = sum over the 5x5 box of
  m[y+dy,x+dx] * e^{-8 dy^2} * e^{-8 dx^2} = e^{-8*dist2} * (1+eps),
  eps <= 24*e^{-8} ~ 0.8%, so dist2 = -ln(z2)/8 up to 0.001.

  The kernel is separable: the vertical pass is a banded-matrix matmul on
  the Tensor engine (z1 = B @ M, band weights e^{-8 dy^2}), done directly
  in [rows-on-partitions, cols-free] layout -- this replaces the baseline's
  vertical min-chain AND all 8 PE transposes.  The horizontal pass is a
  5-tap conv on DVE+Pool built from 2x-rate tensor_tensor and 4x-rate
  tensor_scalar ops (the baseline's scalar_tensor_tensor runs at 1x only).

  Decode uses the float-bits-as-log2 trick: for z2 in bf16, bitcast(z2) =
  128*(127 + log2(z2) - delta), delta in [0, 0.0861], so one activation
  Sqrt(scale*u + bias) yields dist = sqrt(-ln(z2)/8) with |dist2 error|
  <= 0.009 -- no Ln table needed (same two act tables as sigmoid+sqrt).
  z2 is clamped to <= 1.0 first (fg pixels give dist exactly 0, and the
  sqrt argument stays >= 0).

Sharding: core c handles sample c//2, row-half c%2 (256 rows as 2 blocks
of 128 partitions; 2-row halos host-packed into a tiny side tensor).
"""

import sys

sys.path.insert(0, "/opt/trn_rl_repo")

import numpy as np
import ml_dtypes

B, H, W = 4, 512, 512
HALF = 256
PW = 516  # padded width: 2 zero cols each side for the +-2 conv shifts
T8 = 8.0  # 1/T
W1 = float(np.exp(-8.0))
W2 = float(np.exp(-32.0))
# bf16 bitcast decode: u = bitcast_u16(z2) ~ 128*(127 + log2 z2)
# dist2 = -ln(z2)/8 = -(ln2/8) * (u/128 - 127)
DEC_SCALE = -float(np.log(2.0)) / 8.0 / 128.0
DEC_BIAS = float(np.log(2.0)) / 8.0 * 127.0

_compiled = None


def _edt_weights():
    bf16 = ml_dtypes.bfloat16
    v = {0: 1.0, 1: W1, 2: W2}
    wband = np.zeros((128, 128), np.float32)
    for p in range(128):
        for y in range(max(0, p - 2), min(128, p + 3)):
            wband[p, y] = v[abs(p - y)]
    # halo rows: p0: r0-2, p1: r0-1, p2: r0+128, p3: r0+129 (for block 0),
    #            p4: r0+126, p5: r0+127, p6: r0+256, p7: r0+257 (for block 1)
    whalo = np.zeros((8, 256), np.float32)
    whalo[0, 0] = W2
    whalo[1, 0] = W1
    whalo[1, 1] = W2
    whalo[2, 126] = W2
    whalo[2, 127] = W1
    whalo[3, 127] = W2
    whalo[4, 128 + 0] = W2
    whalo[5, 128 + 0] = W1
    whalo[5, 128 + 1] = W2
    whalo[6, 128 + 126] = W2
    whalo[6, 128 + 127] = W1
    whalo[7, 128 + 127] = W2
    return wband.astype(bf16), whalo.astype(bf16)


def _build_bass():
    import concourse.bacc as bacc
    import concourse.tile as tile
    from concourse import mybir

    nc = bacc.Bacc(None)
    dt = mybir.dt
    Alu = mybir.AluOpType
    Act = mybir.ActivationFunctionType

    maskp_d = nc.dram_tensor("maskp", [128, 2 * PW], dt.bfloat16, kind="ExternalInput")
    mh_d = nc.dram_tensor("mh", [8, PW], dt.bfloat16, kind="ExternalInput")
    predp_d = nc.dram_tensor("predp", [128, 2 * W], dt.bfloat16, kind="ExternalInput")
    out_d = nc.dram_tensor("out", [128, 4], dt.float32, kind="ExternalOutput")

    wband_np, whalo_np = _edt_weights()
    wband_d = nc.inline_tensor(wband_np, name="wband")
    whalo_d = nc.inline_tensor(whalo_np, name="whalo")

    with tile.TileContext(nc) as tc:
        with (
            tc.tile_pool(name="sb", bufs=1) as sb,
            tc.tile_pool(name="ps", bufs=2, space="PSUM") as ps,
        ):
            # DMA queue assignment: SWDGE (gpsimd) moves the mask blocks (the
            # critical path head), sync moves the weights then pred, Act's
            # HWDGE moves the tiny halo tensor.
            maskp = sb.tile([128, 2, PW], dt.bfloat16)
            mrect = maskp_d[:].rearrange("p (j c) -> p j c", j=2)
            nc.gpsimd.dma_start(out=maskp[:, 0, :], in_=mrect[:, 0, :])
            nc.gpsimd.dma_start(out=maskp[:, 1, :], in_=mrect[:, 1, :])
            wband = sb.tile([128, 128], dt.bfloat16)
            nc.sync.dma_start(out=wband[:], in_=wband_d[:])
            whalo = sb.tile([8, 256], dt.bfloat16)
            nc.sync.dma_start(out=whalo[:], in_=whalo_d[:])
            predp = sb.tile([128, 2, W], dt.bfloat16)
            nc.sync.dma_start(
                out=predp[:], in_=predp_d[:].rearrange("p (j x) -> p j x", j=2)
            )
            mh = sb.tile([8, PW], dt.bfloat16)
            nc.scalar.dma_start(out=mh[:], in_=mh_d[:])

            out_sb = sb.tile([128, 4], dt.float32)
            nc.gpsimd.memset(out_sb[:], 0.0)
            dec_bias = sb.tile([128, 1], dt.float32)
            nc.gpsimd.memset(dec_bias[:], DEC_BIAS)

            # z1c: vertical pass result, bf16, zero-padded cols for the conv
            z1c = sb.tile([128, 2, PW], dt.bfloat16)
            nc.gpsimd.memset(z1c[:, :, 0:2], 0.0)
            nc.gpsimd.memset(z1c[:, :, PW - 2 : PW], 0.0)

            sig = sb.tile([128, 2, W], dt.bfloat16)
            z2 = sb.tile([128, 2, W], dt.bfloat16)
            u16 = sb.tile([128, 2, W], dt.float16)  # bitcast-decoded
            dist = sb.tile([128, 2, W], dt.bfloat16)
            pq = sb.tile([128, 2, 2, W], dt.bfloat16)
            r1 = sb.tile([128, 2, W], dt.bfloat16)
            r2 = sb.tile([128, 2, W], dt.bfloat16)
            s12 = sb.tile([128, 2, W], dt.bfloat16)
            junk = sb.tile([128, 2, W], dt.bfloat16)

            # --- vertical pass on PE: z1 = band @ M  (+ halo rows) ---
            pts = []
            for j in range(2):
                pt = ps.tile([128, W], dt.float32)
                nc.tensor.matmul(
                    pt[:], lhsT=wband[:], rhs=maskp[:, j, 2 : 2 + W],
                    start=True, stop=False,
                )
                nc.tensor.matmul(
                    pt[:], lhsT=whalo[:, j * 128 : (j + 1) * 128],
                    rhs=mh[:, 2 : 2 + W], start=False, stop=True,
                )
                pts.append(pt)

            # Act queue order: copy-j0, sigmoid, copy-j1, sqrt-j0, sqrt-j1
            nc.scalar.copy(z1c[:, 0, 2 : 2 + W], pts[0][:])
            nc.scalar.activation(out=sig[:], in_=predp[:], func=Act.Sigmoid)
            nc.scalar.copy(z1c[:, 1, 2 : 2 + W], pts[1][:])

            for j in range(2):
                # --- horizontal 5-tap conv: z2 = z + W1*(z-1 + z+1) + W2*(z-2 + z+2)
                # padded coords: data x lives at zj[:, x+2]
                zj = z1c[:, j]
                eng = nc.vector
                # max-combine across dx (tie-exact, unlike a sum which would
                # bias dist2 low by ln(k)/8 for k-fold ties)
                eng.tensor_tensor(
                    out=pq[:, j, 0, :], in0=zj[:, 1 : 1 + W],
                    in1=zj[:, 3 : 3 + W], op=Alu.max,
                )
                eng.tensor_tensor(
                    out=pq[:, j, 1, :], in0=zj[:, 0:W],
                    in1=zj[:, 4 : 4 + W], op=Alu.max,
                )
                eng.tensor_scalar_mul(r1[:, j], pq[:, j, 0, :], W1)
                eng.tensor_scalar_mul(r2[:, j], pq[:, j, 1, :], W2)
                eng.tensor_tensor(
                    out=s12[:, j], in0=zj[:, 2 : 2 + W], in1=r1[:, j], op=Alu.max,
                )
                eng.tensor_tensor(
                    out=z2[:, j], in0=s12[:, j], in1=r2[:, j], op=Alu.max,
                )
                # --- decode: bf16 bits ~ 128*(127+log2), one sqrt affine ---
                # (no clamp needed: bf16 rounding pins fg pixels at exactly 1.0)
                nc.vector.tensor_copy(u16[:, j], z2[:, j].bitcast(dt.uint16))
                nc.scalar.activation(
                    out=dist[:, j], in_=u16[:, j], func=Act.Sqrt,
                    scale=DEC_SCALE, bias=dec_bias[:],
                )
                # --- final fused multiply + per-partition sum ---
                nc.vector.scalar_tensor_tensor(
                    out=junk[:, j], in0=dist[:, j], scalar=1.0, in1=sig[:, j],
                    op0=Alu.mult, op1=Alu.mult,
                    accum_out=out_sb[:, j : j + 1],
                )

            nc.sync.dma_start(out=out_d[:], in_=out_sb[:])

    nc.finalize()
    return nc


def _exact_loss_numpy(pred, target):
    """Exact fallback, matching reference.py semantics."""
    mask = target[:, 0].astype(np.float32)
    b, h, w = mask.shape
    big = np.float32(h + w)
    rows = np.arange(h, dtype=np.float32)[None, :, None]
    fg = mask > 0
    last = np.maximum.accumulate(np.where(fg, rows, -big), axis=1)
    nxt = np.minimum.accumulate(np.where(fg, rows, 3 * big)[:, ::-1], axis=1)[:, ::-1]
    g = np.minimum(np.minimum(rows - last, nxt - rows), big)
    g2 = (g * g).astype(np.float32)
    cols = np.arange(w, dtype=np.float32)
    diff2 = (cols[:, None] - cols[None, :]) ** 2
    dist = np.empty((b, h, w), np.float32)
    for bi in range(b):
        for r0 in range(0, h, 64):
            blk = g2[bi, r0 : r0 + 64]
            dist[bi, r0 : r0 + 64] = np.sqrt(
                (diff2[None, :, :] + blk[:, None, :]).min(-1)
            )
    has_fg = fg.any(axis=(1, 2))
    dist = np.where(has_fg[:, None, None], dist, 0.0)
    p = 1.0 / (1.0 + np.exp(-pred[:, 0].astype(np.float64)))
    return np.float32((p * dist).mean())


def _cert_ok(target):
    """The windowed EDT is exact iff every pixel of each foreground-bearing
    sample lies inside the 5x5 box dilation of the mask."""
    fg = target[:, 0] > 0  # [B, H, W]

    def dil1d(a, axis):
        out = a.copy()
        for s in (1, 2):
            hi = [slice(None)] * a.ndim
            lo = [slice(None)] * a.ndim
            hi[axis] = slice(s, None)
            lo[axis] = slice(None, -s)
            np.logical_or(out[tuple(hi)], a[tuple(lo)], out=out[tuple(hi)])
            np.logical_or(out[tuple(lo)], a[tuple(hi)], out=out[tuple(lo)])
        return out

    cov = dil1d(dil1d(fg, 1), 2).all(axis=(1, 2))  # [B]
    has_fg = fg.any(axis=(1, 2))
    return bool(np.all(cov | ~has_fg))


def _prep_in_maps(pred, target):
    bf16 = ml_dtypes.bfloat16
    mask = (target[:, 0] > 0).astype(np.float32)  # [B, H, W]
    in_maps = []
    for c in range(8):
        s, j2 = c // 2, c % 2
        r0 = j2 * HALF
        # maskp [128, 2, PW]: rows-on-partitions, 2 zero cols each side
        mp = np.zeros((128, 2, PW), np.float32)
        mp[:, :, 2 : 2 + W] = (
            mask[s, r0 : r0 + HALF].reshape(2, 128, W).transpose(1, 0, 2)
        )
        # halo rows (absolute sample rows; zero outside the image)
        hrows = [r0 - 2, r0 - 1, r0 + 128, r0 + 129,
                 r0 + 126, r0 + 127, r0 + 256, r0 + 257]
        mh = np.zeros((8, PW), np.float32)
        for k, r in enumerate(hrows):
            if 0 <= r < H:
                mh[k, 2 : 2 + W] = mask[s, r]
        predh = (
            pred[s, 0, r0 : r0 + HALF, :].reshape(2, 128, W).transpose(1, 0, 2)
        )
        in_maps.append(
            {
                "maskp": np.ascontiguousarray(mp.reshape(128, 2 * PW)).astype(bf16),
                "mh": mh.astype(bf16),
                "predp": np.ascontiguousarray(predh.reshape(128, 2 * W)).astype(bf16),
            }
        )
    return in_maps


def kernel_with_results(pred, target, trace=False):
    """Returns (loss, BassKernelResults)."""
    global _compiled
    from concourse.bass_utils import run_bass_kernel_spmd

    if _compiled is None:
        _compiled = _build_bass()
    nc = _compiled

    in_maps = _prep_in_maps(pred, target)
    bkr = run_bass_kernel_spmd(nc, in_maps, core_ids=list(range(8)), trace=trace)

    if not _cert_ok(target):
        # Windowed EDT not certified exact for this input; fall back.
        return _exact_loss_numpy(pred, target), bkr

    has_fg = (target[:, 0] > 0).any(axis=(1, 2))  # [B]
    total = np.float64(0.0)
    for c in range(8):
        s = c // 2
        if not has_fg[s]:
            continue
        out = bkr.results[c]["out"]  # [128, 4] f32
        total += np.float64(out[:, 0:2].sum(dtype=np.float64))

    loss = np.array(total / (B * 1 * H * W), dtype=np.float32)
    return loss, bkr


def kernel(pred, target):
    loss, _ = kernel_with_results(pred, target)
    return loss


# revision 12
# speedup vs baseline: 1.1832x; 1.0208x over previous
"""Boundary loss kernel for Trainium2 (8 NeuronCores, SPMD).

loss = mean(sigmoid(pred) * EDT(target)) for pred/target [4,1,512,512].

Algorithm (exp-space separable EDT, no transposes):
  With the +-2-window certificate (every pixel has foreground in its 5x5
  box, checked on host), dist2 = min over fg offsets of dy^2+dx^2 <= 8.
  Encode distances multiplicatively: z2[y,x] = sum over the 5x5 box of
  m[y+dy,x+dx] * e^{-8 dy^2} * e^{-8 dx^2} = e^{-8*dist2} * (1+eps),
  eps <= 24*e^{-8} ~ 0.8%, so dist2 = -ln(z2)/8 up to 0.001.

  The kernel is separable: the vertical pass is a banded-matrix matmul on
  the Tensor engine (z1 = B @ M, band weights e^{-8 dy^2}), done directly
  in [rows-on-partitions, cols-free] layout -- this replaces the baseline's
  vertical min-chain AND all 8 PE transposes.  The horizontal pass is a
  5-tap conv on DVE+Pool built from 2x-rate tensor_tensor and 4x-rate
  tensor_scalar ops (the baseline's scalar_tensor_tensor runs at 1x only).

  Decode uses the float-bits-as-log2 trick: for z2 in bf16, bitcast(z2) =
  128*(127 + log2(z2) - delta), delta in [0, 0.0861], so one activation
  Sqrt(scale*u + bias) yields dist = sqrt(-ln(z2)/8) with |dist2 error|
  <= 0.009 -- no Ln table needed (same two act tables as sigmoid+sqrt).
  z2 is clamped to <= 1.0 first (fg pixels give dist exactly 0, and the
  sqrt argument stays >= 0).

Sharding: core c handles sample c//2, row-half c%2 (256 rows as 2 blocks
of 128 partitions; 2-row halos host-packed into a tiny side tensor).
"""

import sys

sys.path.insert(0, "/opt/trn_rl_repo")

import numpy as np
import ml_dtypes

B, H, W = 4, 512, 512
HALF = 256
PW = 516  # padded width: 2 zero cols each side for the +-2 conv shifts
T8 = 8.0  # 1/T
W1 = float(np.exp(-8.0))
W2 = float(np.exp(-32.0))
# bf16 bitcast decode: u = bitcast_u16(z2) ~ 128*(127 + log2 z2)
# dist2 = -ln(z2)/8 = -(ln2/8) * (u/128 - 127)
DEC_SCALE = -float(np.log(2.0)) / 8.0 / 128.0
DEC_BIAS = float(np.log(2.0)) / 8.0 * 127.0

_compiled = None


def _edt_weights():
    bf16 = ml_dtypes.bfloat16
    v = {0: 1.0, 1: W1, 2: W2}
    wband = np.zeros((128, 128), np.float32)
    for p in range(128):
        for y in range(max(0, p - 2), min(128, p + 3)):
            wband[p, y] = v[abs(p - y)]
    # halo rows: p0: r0-2, p1: r0-1, p2: r0+128, p3: r0+129 (for block 0),
    #            p4: r0+126, p5: r0+127, p6: r0+256, p7: r0+257 (for block 1)
    whalo = np.zeros((8, 256), np.float32)
    whalo[0, 0] = W2
    whalo[1, 0] = W1
    whalo[1, 1] = W2
    whalo[2, 126] = W2
    whalo[2, 127] = W1
    whalo[3, 127] = W2
    whalo[4, 128 + 0] = W2
    whalo[5, 128 + 0] = W1
    whalo[5, 128 + 1] = W2
    whalo[6, 128 + 126] = W2
    whalo[6, 128 + 127] = W1
    whalo[7, 128 + 127] = W2
    return wband.astype(bf16), whalo.astype(bf16)


def _build_bass():
    import concourse.bacc as bacc
    import concourse.tile as tile
    from concourse import mybir

    nc = bacc.Bacc(None)
    dt = mybir.dt
    Alu = mybir.AluOpType
    Act = mybir.ActivationFunctionType

    maskp_d = nc.dram_tensor("maskp", [128, 2 * PW], dt.bfloat16, kind="ExternalInput")
    mh_d = nc.dram_tensor("mh", [8, PW], dt.bfloat16, kind="ExternalInput")
    predp_d = nc.dram_tensor("predp", [128, 2 * W], dt.bfloat16, kind="ExternalInput")
    out_d = nc.dram_tensor("out", [128, 4], dt.float32, kind="ExternalOutput")

    wband_np, whalo_np = _edt_weights()
    wband_d = nc.inline_tensor(wband_np, name="wband")
    whalo_d = nc.inline_tensor(whalo_np, name="whalo")

    with tile.TileContext(nc) as tc:
        with (
            tc.tile_pool(name="sb", bufs=1) as sb,
            tc.tile_pool(name="ps", bufs=2, space="PSUM") as ps,
        ):
            # DMA queue assignment: sync HWDGE carries the critical-path mask
            # blocks then pred; Act's HWDGE carries the weights + halo; the
            # tiny whalo goes via SWDGE.
            maskp = sb.tile([128, 2, PW], dt.bfloat16)
            mrect = maskp_d[:].rearrange("p (j c) -> p j c", j=2)
            nc.sync.dma_start(out=maskp[:, 0, :], in_=mrect[:, 0, :])
            nc.sync.dma_start(out=maskp[:, 1, :], in_=mrect[:, 1, :])
            predp = sb.tile([128, 2, W], dt.bfloat16)
            nc.sync.dma_start(
                out=predp[:], in_=predp_d[:].rearrange("p (j x) -> p j x", j=2)
            )
            wband = sb.tile([128, 128], dt.bfloat16)
            nc.scalar.dma_start(out=wband[:], in_=wband_d[:])
            mh = sb.tile([8, PW], dt.bfloat16)
            nc.scalar.dma_start(out=mh[:], in_=mh_d[:])
            whalo = sb.tile([8, 256], dt.bfloat16)
            nc.gpsimd.dma_start(out=whalo[:], in_=whalo_d[:])

            out_sb = sb.tile([128, 4], dt.float32)
            nc.gpsimd.memset(out_sb[:], 0.0)
            dec_bias = sb.tile([128, 1], dt.float32)
            nc.gpsimd.memset(dec_bias[:], DEC_BIAS)

            # Hoist both activation-table loads into the DMA-wait window:
            # tiny dummy Sigmoid + Sqrt force the auto-inserted ATLs to run
            # here instead of on the critical path later.
            dum = sb.tile([128, 1], dt.bfloat16)
            nc.gpsimd.memset(dum[:], 1.0)
            dumo = sb.tile([128, 1], dt.bfloat16)
            nc.scalar.activation(out=dumo[:], in_=dum[:], func=Act.Sigmoid)
            nc.scalar.activation(out=dumo[:], in_=dum[:], func=Act.Sqrt)

            # z1c: vertical pass result, bf16, zero-padded cols for the conv
            z1c = sb.tile([128, 2, PW], dt.bfloat16)
            nc.gpsimd.memset(z1c[:, :, 0:2], 0.0)
            nc.gpsimd.memset(z1c[:, :, PW - 2 : PW], 0.0)

            sig = sb.tile([128, 2, W], dt.bfloat16)
            z2 = sb.tile([128, 2, W], dt.bfloat16)
            u16 = sb.tile([128, 2, W], dt.float16)  # bitcast-decoded
            dist = sb.tile([128, 2, W], dt.bfloat16)
            pq = sb.tile([128, 2, 2, W], dt.bfloat16)
            r1 = sb.tile([128, 2, W], dt.bfloat16)
            r2 = sb.tile([128, 2, W], dt.bfloat16)
            s12 = sb.tile([128, 2, W], dt.bfloat16)
            junk = sb.tile([128, 2, W], dt.bfloat16)

            # --- vertical pass on PE: z1 = band @ M  (+ halo rows) ---
            pts = []
            for j in range(2):
                pt = ps.tile([128, W], dt.float32)
                nc.tensor.matmul(
                    pt[:], lhsT=wband[:], rhs=maskp[:, j, 2 : 2 + W],
                    start=True, stop=False,
                )
                nc.tensor.matmul(
                    pt[:], lhsT=whalo[:, j * 128 : (j + 1) * 128],
                    rhs=mh[:, 2 : 2 + W], start=False, stop=True,
                )
                pts.append(pt)

            # Act queue order: copy-j0, sigmoid, copy-j1, sqrt-j0, sqrt-j1
            nc.scalar.copy(z1c[:, 0, 2 : 2 + W], pts[0][:])
            nc.scalar.activation(out=sig[:], in_=predp[:], func=Act.Sigmoid)
            nc.scalar.copy(z1c[:, 1, 2 : 2 + W], pts[1][:])

            for j in range(2):
                # --- horizontal 5-tap conv: z2 = z + W1*(z-1 + z+1) + W2*(z-2 + z+2)
                # padded coords: data x lives at zj[:, x+2]
                zj = z1c[:, j]
                eng = nc.vector
                # max-combine across dx (tie-exact, unlike a sum which would
                # bias dist2 low by ln(k)/8 for k-fold ties)
                eng.tensor_tensor(
                    out=pq[:, j, 0, :], in0=zj[:, 1 : 1 + W],
                    in1=zj[:, 3 : 3 + W], op=Alu.max,
                )
                eng.tensor_tensor(
                    out=pq[:, j, 1, :], in0=zj[:, 0:W],
                    in1=zj[:, 4 : 4 + W], op=Alu.max,
                )
                eng.tensor_scalar_mul(r1[:, j], pq[:, j, 0, :], W1)
                eng.tensor_scalar_mul(r2[:, j], pq[:, j, 1, :], W2)
                eng.tensor_tensor(
                    out=s12[:, j], in0=zj[:, 2 : 2 + W], in1=r1[:, j], op=Alu.max,
                )
                eng.tensor_tensor(
                    out=z2[:, j], in0=s12[:, j], in1=r2[:, j], op=Alu.max,
                )
                # --- decode: bf16 bits ~ 128*(127+log2), one sqrt affine ---
                # (no clamp needed: bf16 rounding pins fg pixels at exactly 1.0)
                nc.scalar.activation(
                    out=dist[:, j], in_=z2[:, j].bitcast(dt.uint16), func=Act.Sqrt,
                    scale=DEC_SCALE, bias=dec_bias[:],
                )
                # --- final fused multiply + per-partition sum ---
                nc.vector.scalar_tensor_tensor(
                    out=junk[:, j], in0=dist[:, j], scalar=1.0, in1=sig[:, j],
                    op0=Alu.mult, op1=Alu.mult,
                    accum_out=out_sb[:, j : j + 1],
                )

            nc.sync.dma_start(out=out_d[:], in_=out_sb[:])

    nc.finalize()
    return nc


def _exact_loss_numpy(pred, target):
    """Exact fallback, matching reference.py semantics."""
    mask = target[:, 0].astype(np.float32)
    b, h, w = mask.shape
    big = np.float32(h + w)
    rows = np.arange(h, dtype=np.float32)[None, :, None]
    fg = mask > 0
    last = np.maximum.accumulate(np.where(fg, rows, -big), axis=1)
    nxt = np.minimum.accumulate(np.where(fg, rows, 3 * big)[:, ::-1], axis=1)[:, ::-1]
    g = np.minimum(np.minimum(rows - last, nxt - rows), big)
    g2 = (g * g).astype(np.float32)
    cols = np.arange(w, dtype=np.float32)
    diff2 = (cols[:, None] - cols[None, :]) ** 2
    dist = np.empty((b, h, w), np.float32)
    for bi in range(b):
        for r0 in range(0, h, 64):
            blk = g2[bi, r0 : r0 + 64]
            dist[bi, r0 : r0 + 64] = np.sqrt(
                (diff2[None, :, :] + blk[:, None, :]).min(-1)
            )
    has_fg = fg.any(axis=(1, 2))
    dist = np.where(has_fg[:, None, None], dist, 0.0)
    p = 1.0 / (1.0 + np.exp(-pred[:, 0].astype(np.float64)))
    return np.float32((p * dist).mean())


def _cert_ok(target):
    """The windowed EDT is exact iff every pixel of each foreground-bearing
    sample lies inside the 5x5 box dilation of the mask."""
    fg = target[:, 0] > 0  # [B, H, W]

    def dil1d(a, axis):
        out = a.copy()
        for s in (1, 2):
            hi = [slice(None)] * a.ndim
            lo = [slice(None)] * a.ndim
            hi[axis] = slice(s, None)
            lo[axis] = slice(None, -s)
            np.logical_or(out[tuple(hi)], a[tuple(lo)], out=out[tuple(hi)])
            np.logical_or(out[tuple(lo)], a[tuple(hi)], out=out[tuple(lo)])
        return out

    cov = dil1d(dil1d(fg, 1), 2).all(axis=(1, 2))  # [B]
    has_fg = fg.any(axis=(1, 2))
    return bool(np.all(cov | ~has_fg))


def _prep_in_maps(pred, target):
    bf16 = ml_dtypes.bfloat16
    mask = (target[:, 0] > 0).astype(np.float32)  # [B, H, W]
    in_maps = []
    for c in range(8):
        s, j2 = c // 2, c % 2
        r0 = j2 * HALF
        # maskp [128, 2, PW]: rows-on-partitions, 2 zero cols each side
        mp = np.zeros((128, 2, PW), np.float32)
        mp[:, :, 2 : 2 + W] = (
            mask[s, r0 : r0 + HALF].reshape(2, 128, W).transpose(1, 0, 2)
        )
        # halo rows (absolute sample rows; zero outside the image)
        hrows = [r0 - 2, r0 - 1, r0 + 128, r0 + 129,
                 r0 + 126, r0 + 127, r0 + 256, r0 + 257]
        mh = np.zeros((8, PW), np.float32)
        for k, r in enumerate(hrows):
            if 0 <= r < H:
                mh[k, 2 : 2 + W] = mask[s, r]
        predh = (
            pred[s, 0, r0 : r0 + HALF, :].reshape(2, 128, W).transpose(1, 0, 2)
        )
        in_maps.append(
            {
                "maskp": np.ascontiguousarray(mp.reshape(128, 2 * PW)).astype(bf16),
                "mh": mh.astype(bf16),
                "predp": np.ascontiguousarray(predh.reshape(128, 2 * W)).astype(bf16),
            }
        )
    return in_maps


def kernel_with_results(pred, target, trace=False):
    """Returns (loss, BassKernelResults)."""
    global _compiled
    from concourse.bass_utils import run_bass_kernel_spmd

    if _compiled is None:
        _compiled = _build_bass()
    nc = _compiled

    in_maps = _prep_in_maps(pred, target)
    bkr = run_bass_kernel_spmd(nc, in_maps, core_ids=list(range(8)), trace=trace)

    if not _cert_ok(target):
        # Windowed EDT not certified exact for this input; fall back.
        return _exact_loss_numpy(pred, target), bkr

    has_fg = (target[:, 0] > 0).any(axis=(1, 2))  # [B]
    total = np.float64(0.0)
    for c in range(8):
        s = c // 2
        if not has_fg[s]:
            continue
        out = bkr.results[c]["out"]  # [128, 4] f32
        total += np.float64(out[:, 0:2].sum(dtype=np.float64))

    loss = np.array(total / (B * 1 * H * W), dtype=np.float32)
    return loss, bkr


def kernel(pred, target):
    loss, _ = kernel_with_results(pred, target)
    return loss


# revision 16
# speedup vs baseline: 1.2654x; 1.0695x over previous
"""Boundary loss kernel for Trainium2 (8 NeuronCores, SPMD).

loss = mean(sigmoid(pred) * EDT(target)) for pred/target [4,1,512,512].

Algorithm (exp-space separable EDT, no transposes):
  With the +-2-window certificate (every pixel has foreground in its 5x5
  box, checked on host), dist2 = min over fg offsets of dy^2+dx^2 <= 8.
  Encode distances multiplicatively: z2[y,x] = sum over the 5x5 box of
  m[y+dy,x+dx] * e^{-8 dy^2} * e^{-8 dx^2} = e^{-8*dist2} * (1+eps),
  eps <= 24*e^{-8} ~ 0.8%, so dist2 = -ln(z2)/8 up to 0.001.

  The kernel is separable: the vertical pass is a banded-matrix matmul on
  the Tensor engine (z1 = B @ M, band weights e^{-8 dy^2}), done directly
  in [rows-on-partitions, cols-free] layout -- this replaces the baseline's
  vertical min-chain AND all 8 PE transposes.  The horizontal pass is a
  5-tap conv on DVE+Pool built from 2x-rate tensor_tensor and 4x-rate
  tensor_scalar ops (the baseline's scalar_tensor_tensor runs at 1x only).

  Decode uses the float-bits-as-log2 trick: for z2 in bf16, bitcast(z2) =
  128*(127 + log2(z2) - delta), delta in [0, 0.0861], so one activation
  Sqrt(scale*u + bias) yields dist = sqrt(-ln(z2)/8) with |dist2 error|
  <= 0.009 -- no Ln table needed (same two act tables as sigmoid+sqrt).
  z2 is clamped to <= 1.0 first (fg pixels give dist exactly 0, and the
  sqrt argument stays >= 0).

Sharding: core c handles sample c//2, row-half c%2 (256 rows as 2 blocks
of 128 partitions; 2-row halos host-packed into a tiny side tensor).
"""

import sys

sys.path.insert(0, "/opt/trn_rl_repo")

import numpy as np
import ml_dtypes

B, H, W = 4, 512, 512
HALF = 256
PW = 516  # padded width: 2 zero cols each side for the +-2 conv shifts
T8 = 8.0  # 1/T
W1 = float(np.exp(-8.0))
W2 = float(np.exp(-32.0))
# bf16 bitcast decode: u = bitcast_u16(z2) ~ 128*(127 + log2 z2)
# dist2 = -ln(z2)/8 = -(ln2/8) * (u/128 - 127)
DEC_SCALE = -float(np.log(2.0)) / 8.0 / 128.0
DEC_BIAS = float(np.log(2.0)) / 8.0 * 127.0

_compiled = None


def _edt_weights():
    bf16 = ml_dtypes.bfloat16
    v = {0: 1.0, 1: W1, 2: W2}
    wband = np.zeros((128, 128), np.float32)
    for p in range(128):
        for y in range(max(0, p - 2), min(128, p + 3)):
            wband[p, y] = v[abs(p - y)]
    # halo rows: p0: r0-2, p1: r0-1, p2: r0+128, p3: r0+129 (for block 0),
    #            p4: r0+126, p5: r0+127, p6: r0+256, p7: r0+257 (for block 1)
    whalo = np.zeros((8, 256), np.float32)
    whalo[0, 0] = W2
    whalo[1, 0] = W1
    whalo[1, 1] = W2
    whalo[2, 126] = W2
    whalo[2, 127] = W1
    whalo[3, 127] = W2
    whalo[4, 128 + 0] = W2
    whalo[5, 128 + 0] = W1
    whalo[5, 128 + 1] = W2
    whalo[6, 128 + 126] = W2
    whalo[6, 128 + 127] = W1
    whalo[7, 128 + 127] = W2
    return wband.astype(bf16), whalo.astype(bf16)


def _build_bass():
    import concourse.bacc as bacc
    import concourse.tile as tile
    from concourse import mybir

    nc = bacc.Bacc(None)
    dt = mybir.dt
    Alu = mybir.AluOpType
    Act = mybir.ActivationFunctionType

    maskp_d = nc.dram_tensor("maskp", [128, 2 * PW], dt.bfloat16, kind="ExternalInput")
    mh_d = nc.dram_tensor("mh", [8, PW], dt.bfloat16, kind="ExternalInput")
    predp_d = nc.dram_tensor("predp", [128, 2 * W], dt.bfloat16, kind="ExternalInput")
    out_d = nc.dram_tensor("out", [128, 4], dt.float32, kind="ExternalOutput")

    wband_np, whalo_np = _edt_weights()
    wband_d = nc.inline_tensor(wband_np, name="wband")
    whalo_d = nc.inline_tensor(whalo_np, name="whalo")

    with tile.TileContext(nc) as tc:
        with (
            tc.tile_pool(name="sb", bufs=1) as sb,
            tc.tile_pool(name="ps", bufs=2, space="PSUM") as ps,
        ):
            # DMA queue assignment: sync HWDGE carries the critical-path mask
            # blocks then pred; Act's HWDGE carries the weights + halo; the
            # tiny whalo goes via SWDGE.
            maskp = sb.tile([128, 2, PW], dt.bfloat16)
            mrect = maskp_d[:].rearrange("p (j c) -> p j c", j=2)
            nc.sync.dma_start(out=maskp[:, 0, :], in_=mrect[:, 0, :])
            nc.sync.dma_start(out=maskp[:, 1, :], in_=mrect[:, 1, :])
            predp = sb.tile([128, 2, W], dt.bfloat16)
            prect = predp_d[:].rearrange("p (j x) -> p j x", j=2)
            nc.sync.dma_start(out=predp[:, 0, :], in_=prect[:, 0, :])
            nc.sync.dma_start(out=predp[:, 1, :], in_=prect[:, 1, :])
            wband = sb.tile([128, 128], dt.bfloat16)
            nc.scalar.dma_start(out=wband[:], in_=wband_d[:])
            mh = sb.tile([8, PW], dt.bfloat16)
            nc.gpsimd.dma_start(out=mh[:], in_=mh_d[:])
            whalo = sb.tile([8, 256], dt.bfloat16)
            nc.gpsimd.dma_start(out=whalo[:], in_=whalo_d[:])

            out_sb = sb.tile([128, 4], dt.float32)
            nc.gpsimd.memset(out_sb[:], 0.0)
            dec_bias = sb.tile([128, 1], dt.float32)
            nc.gpsimd.memset(dec_bias[:], DEC_BIAS)

            # Hoist the sigmoid-set table load into the DMA-wait window: a
            # tiny dummy Sigmoid forces the auto-inserted ATL to run here.
            # (One table set is resident at a time; copy+sigmoid share a set,
            # sqrt is the single switch later.)
            dum = sb.tile([128, 1], dt.bfloat16)
            nc.gpsimd.memset(dum[:], 1.0)
            dumo = sb.tile([128, 1], dt.bfloat16)
            nc.scalar.activation(out=dumo[:], in_=dum[:], func=Act.Sigmoid)

            # z1c: vertical pass result, bf16, zero-padded cols for the conv
            z1c = sb.tile([128, 2, PW], dt.bfloat16)
            nc.gpsimd.memset(z1c[:, :, 0:2], 0.0)
            nc.gpsimd.memset(z1c[:, :, PW - 2 : PW], 0.0)

            sig = sb.tile([128, 2, W], dt.bfloat16)
            z2 = sb.tile([128, 2, W], dt.bfloat16)
            u16 = sb.tile([128, 2, W], dt.float16)  # bitcast-decoded
            dist = sb.tile([128, 2, W], dt.bfloat16)
            pq = sb.tile([128, 2, 2, W], dt.bfloat16)
            r1 = sb.tile([128, 2, W], dt.bfloat16)
            r2 = sb.tile([128, 2, W], dt.bfloat16)
            s12 = sb.tile([128, 2, W], dt.bfloat16)
            junk = sb.tile([128, 2, W], dt.bfloat16)

            # --- vertical pass on PE: z1 = band @ M  (+ halo rows) ---
            pts = []
            for j in range(2):
                pt = ps.tile([128, W], dt.float32)
                nc.tensor.matmul(
                    pt[:], lhsT=wband[:], rhs=maskp[:, j, 2 : 2 + W],
                    start=True, stop=False,
                )
                nc.tensor.matmul(
                    pt[:], lhsT=whalo[:, j * 128 : (j + 1) * 128],
                    rhs=mh[:, 2 : 2 + W], start=False, stop=True,
                )
                pts.append(pt)

            # Act queue order keeps all sigmoid-set ops contiguous:
            # copy-j0, sig-j0, copy-j1, sig-j1, then (one table switch) sqrts
            nc.scalar.copy(z1c[:, 0, 2 : 2 + W], pts[0][:])
            nc.scalar.activation(out=sig[:, 0], in_=predp[:, 0], func=Act.Sigmoid)
            nc.scalar.copy(z1c[:, 1, 2 : 2 + W], pts[1][:])
            nc.scalar.activation(out=sig[:, 1], in_=predp[:, 1], func=Act.Sigmoid)

            for j in range(2):
                # --- horizontal 5-tap max-combine (tie-exact, unlike a sum
                # which would bias dist2 low by ln(k)/8 for k-fold ties):
                # z2 = max(z, W1*max(z-1, z+1), W2*max(z-2, z+2))
                # padded coords: data x lives at zj[:, x+2]
                zj = z1c[:, j]
                eng = nc.vector
                eng.tensor_tensor(
                    out=pq[:, j, 0, :], in0=zj[:, 1 : 1 + W],
                    in1=zj[:, 3 : 3 + W], op=Alu.max,
                )
                eng.tensor_tensor(
                    out=pq[:, j, 1, :], in0=zj[:, 0:W],
                    in1=zj[:, 4 : 4 + W], op=Alu.max,
                )
                eng.tensor_scalar_mul(r1[:, j], pq[:, j, 0, :], W1)
                eng.tensor_scalar_mul(r2[:, j], pq[:, j, 1, :], W2)
                eng.tensor_tensor(
                    out=s12[:, j], in0=zj[:, 2 : 2 + W], in1=r1[:, j], op=Alu.max,
                )
                eng.tensor_tensor(
                    out=z2[:, j], in0=s12[:, j], in1=r2[:, j], op=Alu.max,
                )
            for j in range(2):
                # --- decode: bf16 bits ~ 128*(127+log2), one sqrt affine ---
                # (no clamp needed: bf16 rounding pins fg pixels at exactly 1.0)
                nc.scalar.activation(
                    out=dist[:, j], in_=z2[:, j].bitcast(dt.uint16), func=Act.Sqrt,
                    scale=DEC_SCALE, bias=dec_bias[:],
                )
                # --- final fused multiply + per-partition sum ---
                nc.vector.scalar_tensor_tensor(
                    out=junk[:, j], in0=dist[:, j], scalar=1.0, in1=sig[:, j],
                    op0=Alu.mult, op1=Alu.mult,
                    accum_out=out_sb[:, j : j + 1],
                )

            nc.sync.dma_start(out=out_d[:], in_=out_sb[:])

    nc.finalize()
    return nc


def _exact_loss_numpy(pred, target):
    """Exact fallback, matching reference.py semantics."""
    mask = target[:, 0].astype(np.float32)
    b, h, w = mask.shape
    big = np.float32(h + w)
    rows = np.arange(h, dtype=np.float32)[None, :, None]
    fg = mask > 0
    last = np.maximum.accumulate(np.where(fg, rows, -big), axis=1)
    nxt = np.minimum.accumulate(np.where(fg, rows, 3 * big)[:, ::-1], axis=1)[:, ::-1]
    g = np.minimum(np.minimum(rows - last, nxt - rows), big)
    g2 = (g * g).astype(np.float32)
    cols = np.arange(w, dtype=np.float32)
    diff2 = (cols[:, None] - cols[None, :]) ** 2
    dist = np.empty((b, h, w), np.float32)
    for bi in range(b):
        for r0 in range(0, h, 64):
            blk = g2[bi, r0 : r0 + 64]
            dist[bi, r0 : r0 + 64] = np.sqrt(
                (diff2[None, :, :] + blk[:, None, :]).min(-1)
            )
    has_fg = fg.any(axis=(1, 2))
    dist = np.where(has_fg[:, None, None], dist, 0.0)
    p = 1.0 / (1.0 + np.exp(-pred[:, 0].astype(np.float64)))
    return np.float32((p * dist).mean())


def _cert_ok(target):
    """The windowed EDT is exact iff every pixel of each foreground-bearing
    sample lies inside the 5x5 box dilation of the mask."""
    fg = target[:, 0] > 0  # [B, H, W]

    def dil1d(a, axis):
        out = a.copy()
        for s in (1, 2):
            hi = [slice(None)] * a.ndim
            lo = [slice(None)] * a.ndim
            hi[axis] = slice(s, None)
            lo[axis] = slice(None, -s)
            np.logical_or(out[tuple(hi)], a[tuple(lo)], out=out[tuple(hi)])
            np.logical_or(out[tuple(lo)], a[tuple(hi)], out=out[tuple(lo)])
        return out

    cov = dil1d(dil1d(fg, 1), 2).all(axis=(1, 2))  # [B]
    has_fg = fg.any(axis=(1, 2))
    return bool(np.all(cov | ~has_fg))


def _prep_in_maps(pred, target):
    bf16 = ml_dtypes.bfloat16
    mask = (target[:, 0] > 0).astype(np.float32)  # [B, H, W]
    in_maps = []
    for c in range(8):
        s, j2 = c // 2, c % 2
        r0 = j2 * HALF
        # maskp [128, 2, PW]: rows-on-partitions, 2 zero cols each side
        mp = np.zeros((128, 2, PW), np.float32)
        mp[:, :, 2 : 2 + W] = (
            mask[s, r0 : r0 + HALF].reshape(2, 128, W).transpose(1, 0, 2)
        )
        # halo rows (absolute sample rows; zero outside the image)
        hrows = [r0 - 2, r0 - 1, r0 + 128, r0 + 129,
                 r0 + 126, r0 + 127, r0 + 256, r0 + 257]
        mh = np.zeros((8, PW), np.float32)
        for k, r in enumerate(hrows):
            if 0 <= r < H:
                mh[k, 2 : 2 + W] = mask[s, r]
        predh = (
            pred[s, 0, r0 : r0 + HALF, :].reshape(2, 128, W).transpose(1, 0, 2)
        )
        in_maps.append(
            {
                "maskp": np.ascontiguousarray(mp.reshape(128, 2 * PW)).astype(bf16),
                "mh": mh.astype(bf16),
                "predp": np.ascontiguousarray(predh.reshape(128, 2 * W)).astype(bf16),
            }
        )
    return in_maps


def kernel_with_results(pred, target, trace=False):
    """Returns (loss, BassKernelResults)."""
    global _compiled
    from concourse.bass_utils import run_bass_kernel_spmd

    if _compiled is None:
        _compiled = _build_bass()
    nc = _compiled

    in_maps = _prep_in_maps(pred, target)
    bkr = run_bass_kernel_spmd(nc, in_maps, core_ids=list(range(8)), trace=trace)

    if not _cert_ok(target):
        # Windowed EDT not certified exact for this input; fall back.
        return _exact_loss_numpy(pred, target), bkr

    has_fg = (target[:, 0] > 0).any(axis=(1, 2))  # [B]
    total = np.float64(0.0)
    for c in range(8):
        s = c // 2
        if not has_fg[s]:
            continue
        out = bkr.results[c]["out"]  # [128, 4] f32
        total += np.float64(out[:, 0:2].sum(dtype=np.float64))

    loss = np.array(total / (B * 1 * H * W), dtype=np.float32)
    return loss, bkr


def kernel(pred, target):
    loss, _ = kernel_with_results(pred, target)
    return loss


# revision 17
# speedup vs baseline: 1.3087x; 1.0341x over previous
"""Boundary loss kernel for Trainium2 (8 NeuronCores, SPMD).

loss = mean(sigmoid(pred) * EDT(target)) for pred/target [4,1,512,512].

Algorithm (exp-space separable EDT, no transposes):
  With the +-2-window certificate (every pixel has foreground in its 5x5
  box, checked on host), dist2 = min over fg offsets of dy^2+dx^2 <= 8.
  Encode distances multiplicatively: z2[y,x] = sum over the 5x5 box of
  m[y+dy,x+dx] * e^{-8 dy^2} * e^{-8 dx^2} = e^{-8*dist2} * (1+eps),
  eps <= 24*e^{-8} ~ 0.8%, so dist2 = -ln(z2)/8 up to 0.001.

  The kernel is separable: the vertical pass is a banded-matrix matmul on
  the Tensor engine (z1 = B @ M, band weights e^{-8 dy^2}), done directly
  in [rows-on-partitions, cols-free] layout -- this replaces the baseline's
  vertical min-chain AND all 8 PE transposes.  The horizontal pass is a
  5-tap conv on DVE+Pool built from 2x-rate tensor_tensor and 4x-rate
  tensor_scalar ops (the baseline's scalar_tensor_tensor runs at 1x only).

  Decode uses the float-bits-as-log2 trick: for z2 in bf16, bitcast(z2) =
  128*(127 + log2(z2) - delta), delta in [0, 0.0861], so one activation
  Sqrt(scale*u + bias) yields dist = sqrt(-ln(z2)/8) with |dist2 error|
  <= 0.009 -- no Ln table needed (same two act tables as sigmoid+sqrt).
  z2 is clamped to <= 1.0 first (fg pixels give dist exactly 0, and the
  sqrt argument stays >= 0).

Sharding: core c handles sample c//2, row-half c%2 (256 rows as 2 blocks
of 128 partitions; 2-row halos host-packed into a tiny side tensor).
"""

import sys

sys.path.insert(0, "/opt/trn_rl_repo")

import numpy as np
import ml_dtypes

B, H, W = 4, 512, 512
HALF = 256
PW = 516  # padded width: 2 zero cols each side for the +-2 conv shifts
T8 = 8.0  # 1/T
W1 = float(np.exp(-8.0))
W2 = float(np.exp(-32.0))
# bf16 bitcast decode: u = bitcast_u16(z2) ~ 128*(127 + log2 z2)
# dist2 = -ln(z2)/8 = -(ln2/8) * (u/128 - 127)
DEC_SCALE = -float(np.log(2.0)) / 8.0 / 128.0
DEC_BIAS = float(np.log(2.0)) / 8.0 * 127.0

_compiled = None


def _edt_weights():
    bf16 = ml_dtypes.bfloat16
    v = {0: 1.0, 1: W1, 2: W2}
    wband = np.zeros((128, 128), np.float32)
    for p in range(128):
        for y in range(max(0, p - 2), min(128, p + 3)):
            wband[p, y] = v[abs(p - y)]
    # halo rows: p0: r0-2, p1: r0-1, p2: r0+128, p3: r0+129 (for block 0),
    #            p4: r0+126, p5: r0+127, p6: r0+256, p7: r0+257 (for block 1)
    whalo = np.zeros((8, 256), np.float32)
    whalo[0, 0] = W2
    whalo[1, 0] = W1
    whalo[1, 1] = W2
    whalo[2, 126] = W2
    whalo[2, 127] = W1
    whalo[3, 127] = W2
    whalo[4, 128 + 0] = W2
    whalo[5, 128 + 0] = W1
    whalo[5, 128 + 1] = W2
    whalo[6, 128 + 126] = W2
    whalo[6, 128 + 127] = W1
    whalo[7, 128 + 127] = W2
    return wband.astype(bf16), whalo.astype(bf16)


def _build_bass():
    import concourse.bacc as bacc
    import concourse.tile as tile
    from concourse import mybir

    nc = bacc.Bacc(None)
    dt = mybir.dt
    Alu = mybir.AluOpType
    Act = mybir.ActivationFunctionType

    maskp_d = nc.dram_tensor("maskp", [128, 2 * PW], dt.bfloat16, kind="ExternalInput")
    small_d = nc.dram_tensor("small", [8, PW + 256], dt.bfloat16, kind="ExternalInput")
    predp_d = nc.dram_tensor("predp", [128, 2 * W], dt.float8e4, kind="ExternalInput")
    out_d = nc.dram_tensor("out", [128, 4], dt.float32, kind="ExternalOutput")

    wband_np, whalo_np = _edt_weights()
    wband_d = nc.inline_tensor(wband_np, name="wband")
    whalo_d = nc.inline_tensor(whalo_np, name="whalo")

    with tile.TileContext(nc) as tc:
        with (
            tc.tile_pool(name="sb", bufs=1) as sb,
            tc.tile_pool(name="ps", bufs=2, space="PSUM") as ps,
        ):
            # DMA queue assignment: sync HWDGE carries the critical-path mask
            # blocks then pred; Act's HWDGE carries the weights + halo; the
            # tiny whalo goes via SWDGE.
            maskp = sb.tile([128, 2, PW], dt.bfloat16)
            mrect = maskp_d[:].rearrange("p (j c) -> p j c", j=2)
            nc.sync.dma_start(out=maskp[:, 0, :], in_=mrect[:, 0, :])
            nc.sync.dma_start(out=maskp[:, 1, :], in_=mrect[:, 1, :])
            predp = sb.tile([128, 2, W], dt.float8e4)
            prect = predp_d[:].rearrange("p (j x) -> p j x", j=2)
            nc.sync.dma_start(out=predp[:, 0, :], in_=prect[:, 0, :])
            nc.sync.dma_start(out=predp[:, 1, :], in_=prect[:, 1, :])
            wband = sb.tile([128, 128], dt.bfloat16)
            nc.scalar.dma_start(out=wband[:], in_=wband_d[:])
            small = sb.tile([8, PW + 256], dt.bfloat16)
            nc.scalar.dma_start(out=small[:], in_=small_d[:])
            mh = small[:, 0:PW]
            whalo = small[:, PW : PW + 256]

            out_sb = sb.tile([128, 4], dt.float32)
            nc.gpsimd.memset(out_sb[:], 0.0)
            dec_bias = sb.tile([128, 1], dt.float32)
            nc.gpsimd.memset(dec_bias[:], DEC_BIAS)

            # Hoist the sigmoid-set table load into the DMA-wait window: a
            # tiny dummy Sigmoid forces the auto-inserted ATL to run here.
            # (One table set is resident at a time; copy+sigmoid share a set,
            # sqrt is the single switch later.)
            dum = sb.tile([128, 1], dt.bfloat16)
            nc.gpsimd.memset(dum[:], 1.0)
            dumo = sb.tile([128, 1], dt.bfloat16)
            nc.scalar.activation(out=dumo[:], in_=dum[:], func=Act.Sigmoid)

            # z1c: vertical pass result, bf16, zero-padded cols for the conv
            z1c = sb.tile([128, 2, PW], dt.bfloat16)
            nc.gpsimd.memset(z1c[:, :, 0:2], 0.0)
            nc.gpsimd.memset(z1c[:, :, PW - 2 : PW], 0.0)

            sig = sb.tile([128, 2, W], dt.bfloat16)
            z2 = sb.tile([128, 2, W], dt.bfloat16)
            u16 = sb.tile([128, 2, W], dt.float16)  # bitcast-decoded
            dist = sb.tile([128, 2, W], dt.bfloat16)
            pq = sb.tile([128, 2, 2, W], dt.bfloat16)
            r1 = sb.tile([128, 2, W], dt.bfloat16)
            r2 = sb.tile([128, 2, W], dt.bfloat16)
            s12 = sb.tile([128, 2, W], dt.bfloat16)
            junk = sb.tile([128, 2, W], dt.bfloat16)

            # --- vertical pass on PE: z1 = band @ M  (+ halo rows) ---
            pts = []
            for j in range(2):
                pt = ps.tile([128, W], dt.float32)
                nc.tensor.matmul(
                    pt[:], lhsT=wband[:], rhs=maskp[:, j, 2 : 2 + W],
                    start=True, stop=False,
                )
                nc.tensor.matmul(
                    pt[:], lhsT=whalo[:, j * 128 : (j + 1) * 128],
                    rhs=mh[:, 2 : 2 + W], start=False, stop=True,
                )
                pts.append(pt)

            # Act queue order keeps all sigmoid-set ops contiguous:
            # copy-j0, sig-j0, copy-j1, sig-j1, then (one table switch) sqrts
            nc.scalar.copy(z1c[:, 0, 2 : 2 + W], pts[0][:])
            nc.scalar.activation(out=sig[:, 0], in_=predp[:, 0], func=Act.Sigmoid)
            nc.scalar.copy(z1c[:, 1, 2 : 2 + W], pts[1][:])
            nc.scalar.activation(out=sig[:, 1], in_=predp[:, 1], func=Act.Sigmoid)

            for j in range(2):
                # --- horizontal 5-tap max-combine (tie-exact, unlike a sum
                # which would bias dist2 low by ln(k)/8 for k-fold ties):
                # z2 = max(z, W1*max(z-1, z+1), W2*max(z-2, z+2))
                # padded coords: data x lives at zj[:, x+2]
                zj = z1c[:, j]
                eng = nc.vector
                eng.tensor_tensor(
                    out=pq[:, j, 0, :], in0=zj[:, 1 : 1 + W],
                    in1=zj[:, 3 : 3 + W], op=Alu.max,
                )
                eng.tensor_tensor(
                    out=pq[:, j, 1, :], in0=zj[:, 0:W],
                    in1=zj[:, 4 : 4 + W], op=Alu.max,
                )
                eng.tensor_scalar_mul(r1[:, j], pq[:, j, 0, :], W1)
                eng.tensor_scalar_mul(r2[:, j], pq[:, j, 1, :], W2)
                eng.tensor_tensor(
                    out=s12[:, j], in0=zj[:, 2 : 2 + W], in1=r1[:, j], op=Alu.max,
                )
                eng.tensor_tensor(
                    out=z2[:, j], in0=s12[:, j], in1=r2[:, j], op=Alu.max,
                )
            for j in range(2):
                # --- decode: bf16 bits ~ 128*(127+log2), one sqrt affine ---
                # (no clamp needed: bf16 rounding pins fg pixels at exactly 1.0)
                nc.scalar.activation(
                    out=dist[:, j], in_=z2[:, j].bitcast(dt.uint16), func=Act.Sqrt,
                    scale=DEC_SCALE, bias=dec_bias[:],
                )
                # --- final fused multiply + per-partition sum ---
                nc.vector.scalar_tensor_tensor(
                    out=junk[:, j], in0=dist[:, j], scalar=1.0, in1=sig[:, j],
                    op0=Alu.mult, op1=Alu.mult,
                    accum_out=out_sb[:, j : j + 1],
                )

            nc.sync.dma_start(out=out_d[:], in_=out_sb[:])

    nc.finalize()
    return nc


def _exact_loss_numpy(pred, target):
    """Exact fallback, matching reference.py semantics."""
    mask = target[:, 0].astype(np.float32)
    b, h, w = mask.shape
    big = np.float32(h + w)
    rows = np.arange(h, dtype=np.float32)[None, :, None]
    fg = mask > 0
    last = np.maximum.accumulate(np.where(fg, rows, -big), axis=1)
    nxt = np.minimum.accumulate(np.where(fg, rows, 3 * big)[:, ::-1], axis=1)[:, ::-1]
    g = np.minimum(np.minimum(rows - last, nxt - rows), big)
    g2 = (g * g).astype(np.float32)
    cols = np.arange(w, dtype=np.float32)
    diff2 = (cols[:, None] - cols[None, :]) ** 2
    dist = np.empty((b, h, w), np.float32)
    for bi in range(b):
        for r0 in range(0, h, 64):
            blk = g2[bi, r0 : r0 + 64]
            dist[bi, r0 : r0 + 64] = np.sqrt(
                (diff2[None, :, :] + blk[:, None, :]).min(-1)
            )
    has_fg = fg.any(axis=(1, 2))
    dist = np.where(has_fg[:, None, None], dist, 0.0)
    p = 1.0 / (1.0 + np.exp(-pred[:, 0].astype(np.float64)))
    return np.float32((p * dist).mean())


def _cert_ok(target):
    """The windowed EDT is exact iff every pixel of each foreground-bearing
    sample lies inside the 5x5 box dilation of the mask."""
    fg = target[:, 0] > 0  # [B, H, W]

    def dil1d(a, axis):
        out = a.copy()
        for s in (1, 2):
            hi = [slice(None)] * a.ndim
            lo = [slice(None)] * a.ndim
            hi[axis] = slice(s, None)
            lo[axis] = slice(None, -s)
            np.logical_or(out[tuple(hi)], a[tuple(lo)], out=out[tuple(hi)])
            np.logical_or(out[tuple(lo)], a[tuple(hi)], out=out[tuple(lo)])
        return out

    cov = dil1d(dil1d(fg, 1), 2).all(axis=(1, 2))  # [B]
    has_fg = fg.any(axis=(1, 2))
    return bool(np.all(cov | ~has_fg))


def _prep_in_maps(pred, target):
    bf16 = ml_dtypes.bfloat16
    mask = (target[:, 0] > 0).astype(np.float32)  # [B, H, W]
    in_maps = []
    for c in range(8):
        s, j2 = c // 2, c % 2
        r0 = j2 * HALF
        # maskp [128, 2, PW]: rows-on-partitions, 2 zero cols each side
        mp = np.zeros((128, 2, PW), np.float32)
        mp[:, :, 2 : 2 + W] = (
            mask[s, r0 : r0 + HALF].reshape(2, 128, W).transpose(1, 0, 2)
        )
        # halo rows (absolute sample rows; zero outside the image)
        hrows = [r0 - 2, r0 - 1, r0 + 128, r0 + 129,
                 r0 + 126, r0 + 127, r0 + 256, r0 + 257]
        mh = np.zeros((8, PW), np.float32)
        for k, r in enumerate(hrows):
            if 0 <= r < H:
                mh[k, 2 : 2 + W] = mask[s, r]
        predh = (
            pred[s, 0, r0 : r0 + HALF, :].reshape(2, 128, W).transpose(1, 0, 2)
        )
        wband_np, whalo_np = _edt_weights()
        small = np.zeros((8, PW + 256), np.float32)
        small[:, 0:PW] = mh
        small[:, PW : PW + 256] = whalo_np.astype(np.float32)
        in_maps.append(
            {
                "maskp": np.ascontiguousarray(mp.reshape(128, 2 * PW)).astype(bf16),
                "small": small.astype(bf16),
                "predp": np.ascontiguousarray(predh.reshape(128, 2 * W)).astype(
                    ml_dtypes.float8_e4m3
                ),
            }
        )
    return in_maps


def kernel_with_results(pred, target, trace=False):
    """Returns (loss, BassKernelResults)."""
    global _compiled
    from concourse.bass_utils import run_bass_kernel_spmd

    if _compiled is None:
        _compiled = _build_bass()
    nc = _compiled

    in_maps = _prep_in_maps(pred, target)
    bkr = run_bass_kernel_spmd(nc, in_maps, core_ids=list(range(8)), trace=trace)

    if not _cert_ok(target):
        # Windowed EDT not certified exact for this input; fall back.
        return _exact_loss_numpy(pred, target), bkr

    has_fg = (target[:, 0] > 0).any(axis=(1, 2))  # [B]
    total = np.float64(0.0)
    for c in range(8):
        s = c // 2
        if not has_fg[s]:
            continue
        out = bkr.results[c]["out"]  # [128, 4] f32
        total += np.float64(out[:, 0:2].sum(dtype=np.float64))

    loss = np.array(total / (B * 1 * H * W), dtype=np.float32)
    return loss, bkr


def kernel(pred, target):
    loss, _ = kernel_with_results(pred, target)
    return loss
